# revision 4
# baseline (speedup 1.0000x reference)
"""Channel-wise dense (per-channel GEMM) Trainium2 kernel.

Problem: inputs [B=32, H=32, W=32, C=128], W [C=128, N=1024, N=1024],
b [C=128, N=1024].  For each channel c: y_c = relu(x_c @ W_c + b_c) with
x_c = inputs.reshape(B, N, C)[:, :, c]  ([B, N]).  Output is [B, H, W, C]
with channels reversed.

Sharding: channels split across 8 NeuronCores (16 channels per core).
Host-side prep:
  - x is transposed to lhsT layout [P=128, KC=8, CPC=16, B=32] (bf16) so the
    stationary matmul operand DMAs contiguously.
  - W shard [16, 1024, 1024] cast to bf16 (halves the HBM stream, which is
    the roofline term; fp32 accumulate keeps the dot-product error ~2e-3).
  - b is pre-broadcast to quad layout [4, 128, 1024] fp32.
Device (per core): for each channel, stream W_c from HBM as [128, 8, 1024],
16 accumulating matmuls (lhsT = x_c.T chunk [128, 32], rhs = W chunk
[128, 512]) into PSUM [32, 512], DVE adds bias into a 4-channel [128, 1024]
output tile, ACT applies ReLU, quad tile DMAs out with full 128 partitions.
"""

import os

import numpy as np
import ml_dtypes

import concourse.bass as bass
import concourse.mybir as mybir
import concourse.tile as tile
from concourse import bacc
from concourse.bass_utils import run_bass_kernel_spmd

B, H, WD, C = 32, 32, 32, 128
N = H * WD            # 1024
NCORES = 8
CPC = C // NCORES     # 16 channels per core
P = 128
KC = N // P           # 8 contraction chunks of 128
NQ = CPC // 4         # 4 output quads (4 channels each) per core
HF = N // 512         # 2 free-dim halves per matmul row

MM_DT = mybir.dt.bfloat16
MM_NP = ml_dtypes.bfloat16

_CACHE = {}
LAST_RESULTS = None
LAST_IN_MAPS = None


def _build_nc():
    nc = bacc.Bacc(
        "TRN2",
        target_bir_lowering=False,
        debug=False,
        num_devices=NCORES,
    )
    x_d = nc.dram_tensor("x", [P, KC, CPC, B], MM_DT, kind="ExternalInput")
    w_d = nc.dram_tensor("w", [CPC, N, N], MM_DT, kind="ExternalInput")
    b_d = nc.dram_tensor("b", [NQ, P, N], mybir.dt.float32, kind="ExternalInput")
    y_d = nc.dram_tensor("y", [NQ, P, N], mybir.dt.float32, kind="ExternalOutput")

    with tile.TileContext(nc) as tc:
        with (
            tc.tile_pool(name="xp", bufs=1) as xp,
            tc.tile_pool(name="wp", bufs=3) as wp,
            tc.tile_pool(name="bp", bufs=2) as bp,
            tc.tile_pool(name="op", bufs=2) as op,
            tc.tile_pool(name="ps", bufs=6, space="PSUM") as ps,
        ):
            x_sb = xp.tile([P, KC, CPC, B], MM_DT)
            nc.sync.dma_start(x_sb[:], x_d[:])

            for q in range(NQ):
                b_sb = bp.tile([P, N], mybir.dt.float32, tag="bias")
                nc.sync.dma_start(b_sb[:], b_d[q])
                out_sb = op.tile([P, N], mybir.dt.float32, tag="out")

                for j in range(4):
                    c = q * 4 + j
                    w_sb = wp.tile([P, KC, N], MM_DT, tag="w")
                    nc.sync.dma_start(
                        w_sb[:], w_d[c].rearrange("(kc p) m -> p kc m", p=P)
                    )
                    for h in range(HF):
                        pt = ps.tile([B, 512], mybir.dt.float32, tag="ps")
                        for kc in range(KC):
                            nc.tensor.matmul(
                                pt[:],
                                x_sb[:, kc, c, :],
                                w_sb[:, kc, h * 512 : (h + 1) * 512],
                                start=(kc == 0),
                                stop=(kc == KC - 1),
                            )
                        nc.vector.tensor_add(
                            out_sb[j * B : (j + 1) * B, h * 512 : (h + 1) * 512],
                            pt[:],
                            b_sb[j * B : (j + 1) * B, h * 512 : (h + 1) * 512],
                        )

                nc.scalar.activation(
                    out_sb[:], out_sb[:], mybir.ActivationFunctionType.Relu
                )
                nc.sync.dma_start(y_d[q], out_sb[:])

    nc.compile()
    return nc


def _get_nc():
    if "nc" not in _CACHE:
        _CACHE["nc"] = _build_nc()
    return _CACHE["nc"]


def kernel(inputs: np.ndarray, W: np.ndarray, b: np.ndarray) -> np.ndarray:
    global LAST_RESULTS, LAST_IN_MAPS
    inputs = np.asarray(inputs, dtype=np.float32)
    W = np.asarray(W, dtype=np.float32)
    b = np.asarray(b, dtype=np.float32)

    # x lhsT layout: [p, kc, c, b] = inputs[b, kc*128+p, c]
    x = inputs.reshape(B, N, C)
    xt = np.transpose(x, (1, 2, 0)).reshape(KC, P, C, B).transpose(1, 0, 2, 3)
    xt = np.ascontiguousarray(xt).astype(MM_NP)

    in_maps = []
    for r in range(NCORES):
        cs = slice(r * CPC, (r + 1) * CPC)
        x_core = np.ascontiguousarray(xt[:, :, cs, :])
        w_core = np.ascontiguousarray(W[cs]).astype(MM_NP)
        b_shard = b[cs]  # [16, 1024]
        b_core = np.ascontiguousarray(
            np.broadcast_to(
                b_shard.reshape(NQ, 4, 1, N), (NQ, 4, B, N)
            ).reshape(NQ, P, N)
        )
        in_maps.append({"x": x_core, "w": w_core, "b": b_core})

    nc = _get_nc()
    LAST_IN_MAPS = in_maps
    res = run_bass_kernel_spmd(nc, in_maps, list(range(NCORES)))
    LAST_RESULTS = res

    # Gather: per-core y [NQ, 128, 1024] -> [NQ, 4, B, N]; channel = r*16+q*4+j
    ycm = np.concatenate(
        [np.asarray(res.results[r]["y"]).reshape(CPC, B, N) for r in range(NCORES)],
        axis=0,
    )  # [C, B, N]
    ybcn = ycm.transpose(1, 0, 2)  # [B, C, N]
    out = ybcn.reshape(B, C, H, WD).transpose(0, 2, 3, 1)[..., ::-1]
    return np.ascontiguousarray(out, dtype=np.float32)


# revision 5
# speedup vs baseline: 1.0065x; 1.0065x over previous
"""Channel-wise dense (per-channel GEMM) Trainium2 kernel.

Problem: inputs [B=32, H=32, W=32, C=128], W [C=128, N=1024, N=1024],
b [C=128, N=1024].  For each channel c: y_c = relu(x_c @ W_c + b_c) with
x_c = inputs.reshape(B, N, C)[:, :, c]  ([B, N]).  Output is [B, H, W, C]
with channels reversed.

Sharding: channels split across 8 NeuronCores (16 channels per core).
Host-side prep:
  - x is transposed to lhsT layout [P=128, KC=8, CPC=16, B=32] (bf16) so the
    stationary matmul operand DMAs contiguously.
  - W shard [16, 1024, 1024] cast to bf16 (halves the HBM stream, which is
    the roofline term; fp32 accumulate keeps the dot-product error ~2e-3).
  - b: when nonzero, pre-broadcast to quad layout [4, 128, 1024] fp32; the
    spec fills b with zeros, in which case a bias-free program is built and
    ReLU is fused into the PSUM->SBUF eviction on the vector engine.
Device (per core): for each channel, stream W_c from HBM as [128, 8, 1024]
(~2MB DMAs, deep prefetch), 16 accumulating matmuls (lhsT = x_c.T chunk
[128, 32], rhs = W chunk [128, 512]) into a 2-bank PSUM tile [32, 1024],
evict with relu (or +bias, relu) into a 4-channel [128, 1024] output tile,
quad tiles DMA out with full 128 partitions.
"""

import os

import numpy as np
import ml_dtypes

import concourse.bass as bass
import concourse.mybir as mybir
import concourse.tile as tile
from concourse import bacc
from concourse.bass_utils import run_bass_kernel_spmd

B, H, WD, C = 32, 32, 32, 128
N = H * WD            # 1024
NCORES = 8
CPC = C // NCORES     # 16 channels per core
P = 128
KC = N // P           # 8 contraction chunks of 128
NQ = CPC // 4         # 4 output quads (4 channels each) per core
HF = N // 512         # 2 free-dim halves per matmul row

MM_DT = mybir.dt.bfloat16
MM_NP = ml_dtypes.bfloat16

_CACHE = {}
LAST_RESULTS = None
LAST_IN_MAPS = None


def _build_nc(with_bias: bool):
    nc = bacc.Bacc(
        "TRN2",
        target_bir_lowering=False,
        debug=False,
        num_devices=NCORES,
    )
    x_d = nc.dram_tensor("x", [P, KC, CPC, B], MM_DT, kind="ExternalInput")
    w_d = nc.dram_tensor("w", [CPC, N, N], MM_DT, kind="ExternalInput")
    if with_bias:
        b_d = nc.dram_tensor("b", [NQ, P, N], mybir.dt.float32, kind="ExternalInput")
    y_d = nc.dram_tensor("y", [NQ, P, N], mybir.dt.float32, kind="ExternalOutput")

    with tile.TileContext(nc) as tc:
        with (
            tc.tile_pool(name="xp", bufs=1) as xp,
            tc.tile_pool(name="wp", bufs=6) as wp,
            tc.tile_pool(name="bp", bufs=2) as bp,
            tc.tile_pool(name="op", bufs=2) as op,
            tc.tile_pool(name="ps", bufs=4, space="PSUM") as ps,
        ):
            x_sb = xp.tile([P, KC, CPC, B], MM_DT)
            nc.sync.dma_start(x_sb[:], x_d[:])

            for q in range(NQ):
                if with_bias:
                    b_sb = bp.tile([P, N], mybir.dt.float32, tag="bias")
                    nc.sync.dma_start(b_sb[:], b_d[q])
                out_sb = op.tile([P, N], mybir.dt.float32, tag="out")

                for j in range(4):
                    c = q * 4 + j
                    w_sb = wp.tile([P, KC, N], MM_DT, tag="w")
                    nc.sync.dma_start(
                        w_sb[:], w_d[c].rearrange("(kc p) m -> p kc m", p=P)
                    )
                    pt = ps.tile([B, N], mybir.dt.float32, tag="ps")
                    for h in range(HF):
                        for kc in range(KC):
                            nc.tensor.matmul(
                                pt[:, h * 512 : (h + 1) * 512],
                                x_sb[:, kc, c, :],
                                w_sb[:, kc, h * 512 : (h + 1) * 512],
                                start=(kc == 0),
                                stop=(kc == KC - 1),
                            )
                    oslice = out_sb[j * B : (j + 1) * B, :]
                    if with_bias:
                        nc.vector.tensor_add(
                            oslice, pt[:], b_sb[j * B : (j + 1) * B, :]
                        )
                        nc.scalar.activation(
                            oslice, oslice, mybir.ActivationFunctionType.Relu
                        )
                    else:
                        # relu fused into the PSUM eviction
                        nc.vector.tensor_scalar_max(oslice, pt[:], 0.0)

                nc.sync.dma_start(y_d[q], out_sb[:])

    nc.compile()
    return nc


def _get_nc(with_bias: bool):
    key = ("bias" if with_bias else "nobias",)
    if key not in _CACHE:
        _CACHE[key] = _build_nc(with_bias)
    return _CACHE[key]


def kernel(inputs: np.ndarray, W: np.ndarray, b: np.ndarray) -> np.ndarray:
    global LAST_RESULTS, LAST_IN_MAPS
    inputs = np.asarray(inputs, dtype=np.float32)
    W = np.asarray(W, dtype=np.float32)
    b = np.asarray(b, dtype=np.float32)

    with_bias = bool(np.any(b))

    # x lhsT layout: [p, kc, c, b] = inputs[b, kc*128+p, c]
    x = inputs.reshape(B, N, C)
    xt = np.transpose(x, (1, 2, 0)).reshape(KC, P, C, B).transpose(1, 0, 2, 3)
    xt = np.ascontiguousarray(xt).astype(MM_NP)

    in_maps = []
    for r in range(NCORES):
        cs = slice(r * CPC, (r + 1) * CPC)
        x_core = np.ascontiguousarray(xt[:, :, cs, :])
        w_core = np.ascontiguousarray(W[cs]).astype(MM_NP)
        m = {"x": x_core, "w": w_core}
        if with_bias:
            b_shard = b[cs]  # [16, 1024]
            m["b"] = np.ascontiguousarray(
                np.broadcast_to(
                    b_shard.reshape(NQ, 4, 1, N), (NQ, 4, B, N)
                ).reshape(NQ, P, N)
            )
        in_maps.append(m)

    nc = _get_nc(with_bias)
    LAST_IN_MAPS = in_maps
    res = run_bass_kernel_spmd(nc, in_maps, list(range(NCORES)))
    LAST_RESULTS = res

    # Gather: per-core y [NQ, 128, 1024] -> channel r*16 + q*4 + j, batch bb
    ycm = np.concatenate(
        [np.asarray(res.results[r]["y"]).reshape(CPC, B, N) for r in range(NCORES)],
        axis=0,
    )  # [C, B, N]
    ybcn = ycm.transpose(1, 0, 2)  # [B, C, N]
    out = ybcn.reshape(B, C, H, WD).transpose(0, 2, 3, 1)[..., ::-1]
    return np.ascontiguousarray(out, dtype=np.float32)


# revision 9
# speedup vs baseline: 1.1737x; 1.1661x over previous
"""Channel-wise dense (per-channel GEMM) Trainium2 kernel.

Problem: inputs [B=32, H=32, W=32, C=128], W [C=128, N=1024, N=1024],
b [C=128, N=1024].  For each channel c: y_c = relu(x_c @ W_c + b_c) with
x_c = inputs.reshape(B, N, C)[:, :, c]  ([B, N]).  Output is [B, H, W, C]
with channels reversed.

Sharding: channels split across 8 NeuronCores (16 channels per core).
Host-side prep:
  - x is transposed to lhsT layout [P=128, KC=8, CPC=16, B=32] (bf16) so the
    stationary matmul operand DMAs contiguously.
  - W shard [16, 1024, 1024] cast to bf16 (halves the HBM stream, which is
    the roofline term; fp32 accumulate keeps the dot-product error ~2e-3).
  - b: when nonzero, pre-broadcast to quad layout [4, 128, 1024] fp32; the
    spec fills b with zeros, in which case a bias-free program is built and
    ReLU is fused into the PSUM->SBUF eviction on the vector engine.
Device (per core): for each channel, stream W_c from HBM as [128, 8, 1024]
(~2MB DMAs, deep prefetch), 16 accumulating matmuls (lhsT = x_c.T chunk
[128, 32], rhs = W chunk [128, 512]) into a 2-bank PSUM tile [32, 1024],
evict with relu (or +bias, relu) into a 4-channel [128, 1024] output tile,
quad tiles DMA out with full 128 partitions.
"""

import os

import numpy as np
import ml_dtypes

import concourse.bass as bass
import concourse.mybir as mybir
import concourse.tile as tile
from concourse import bacc
from concourse.bass_utils import run_bass_kernel_spmd

B, H, WD, C = 32, 32, 32, 128
N = H * WD            # 1024
NCORES = 8
CPC = C // NCORES     # 16 channels per core
P = 128
KC = N // P           # 8 contraction chunks of 128
NQ = CPC // 4         # 4 output quads (4 channels each) per core
HF = N // 512         # 2 free-dim halves per matmul row

MM_DT = mybir.dt.bfloat16
MM_NP = ml_dtypes.bfloat16

_CACHE = {}
LAST_RESULTS = None
LAST_IN_MAPS = None


def _build_nc(with_bias: bool):
    nc = bacc.Bacc(
        "TRN2",
        target_bir_lowering=False,
        debug=False,
        num_devices=NCORES,
    )
    x_d = nc.dram_tensor("x", [P, KC, CPC, B], MM_DT, kind="ExternalInput")
    # W pre-transposed on host to [c, p, kc, m]: 16KB contiguous per partition
    w_d = nc.dram_tensor("w", [CPC, P, KC, N], MM_DT, kind="ExternalInput")
    if with_bias:
        b_d = nc.dram_tensor("b", [NQ, P, N], mybir.dt.float32, kind="ExternalInput")
    y_d = nc.dram_tensor("y", [NQ, P, N], mybir.dt.float32, kind="ExternalOutput")

    with tile.TileContext(nc) as tc:
        with (
            tc.tile_pool(name="xp", bufs=1) as xp,
            tc.tile_pool(name="wp", bufs=10) as wp,
            tc.tile_pool(name="bp", bufs=2) as bp,
            tc.tile_pool(name="op", bufs=2) as op,
            tc.tile_pool(name="ps", bufs=4, space="PSUM") as ps,
        ):
            x_sb = xp.tile([P, KC, CPC, B], MM_DT)
            nc.sync.dma_start(x_sb[:], x_d[:])

            for q in range(NQ):
                if with_bias:
                    b_sb = bp.tile([P, N], mybir.dt.float32, tag="bias")
                    nc.sync.dma_start(b_sb[:], b_d[q])
                out_sb = op.tile([P, N], mybir.dt.float32, tag="out")

                for j in range(4):
                    c = q * 4 + j
                    # two kc-half tiles per channel: finer pipeline/tail granularity
                    w_half = []
                    for g in range(2):
                        w_sb = wp.tile([P, KC // 2, N], MM_DT, tag="w")
                        nc.sync.dma_start(
                            w_sb[:], w_d[c][:, g * (KC // 2) : (g + 1) * (KC // 2), :]
                        )
                        w_half.append(w_sb)
                    pt = ps.tile([B, N], mybir.dt.float32, tag="ps")
                    for h in range(HF):
                        for kc in range(KC):
                            nc.tensor.matmul(
                                pt[:, h * 512 : (h + 1) * 512],
                                x_sb[:, kc, c, :],
                                w_half[kc // (KC // 2)][
                                    :, kc % (KC // 2), h * 512 : (h + 1) * 512
                                ],
                                start=(kc == 0),
                                stop=(kc == KC - 1),
                            )
                    oslice = out_sb[j * B : (j + 1) * B, :]
                    if with_bias:
                        nc.vector.tensor_add(
                            oslice, pt[:], b_sb[j * B : (j + 1) * B, :]
                        )
                        nc.scalar.activation(
                            oslice, oslice, mybir.ActivationFunctionType.Relu
                        )
                    else:
                        # relu fused into the PSUM eviction
                        nc.vector.tensor_scalar_max(oslice, pt[:], 0.0)

                nc.sync.dma_start(y_d[q], out_sb[:])

    nc.compile()
    return nc


def _get_nc(with_bias: bool):
    key = ("bias" if with_bias else "nobias",)
    if key not in _CACHE:
        _CACHE[key] = _build_nc(with_bias)
    return _CACHE[key]


def kernel(inputs: np.ndarray, W: np.ndarray, b: np.ndarray) -> np.ndarray:
    global LAST_RESULTS, LAST_IN_MAPS
    inputs = np.asarray(inputs, dtype=np.float32)
    W = np.asarray(W, dtype=np.float32)
    b = np.asarray(b, dtype=np.float32)

    with_bias = bool(np.any(b))

    # x lhsT layout: [p, kc, c, b] = inputs[b, kc*128+p, c]
    x = inputs.reshape(B, N, C)
    xt = np.transpose(x, (1, 2, 0)).reshape(KC, P, C, B).transpose(1, 0, 2, 3)
    xt = np.ascontiguousarray(xt).astype(MM_NP)

    in_maps = []
    for r in range(NCORES):
        cs = slice(r * CPC, (r + 1) * CPC)
        x_core = np.ascontiguousarray(xt[:, :, cs, :])
        # [c, n, m] -> [c, p, kc, m] with n = kc*128 + p
        w_core = np.ascontiguousarray(
            W[cs].reshape(CPC, KC, P, N).transpose(0, 2, 1, 3)
        ).astype(MM_NP)
        m = {"x": x_core, "w": w_core}
        if with_bias:
            b_shard = b[cs]  # [16, 1024]
            m["b"] = np.ascontiguousarray(
                np.broadcast_to(
                    b_shard.reshape(NQ, 4, 1, N), (NQ, 4, B, N)
                ).reshape(NQ, P, N)
            )
        in_maps.append(m)

    nc = _get_nc(with_bias)
    LAST_IN_MAPS = in_maps
    res = run_bass_kernel_spmd(nc, in_maps, list(range(NCORES)))
    LAST_RESULTS = res

    # Gather: per-core y [NQ, 128, 1024] -> channel r*16 + q*4 + j, batch bb
    ycm = np.concatenate(
        [np.asarray(res.results[r]["y"]).reshape(CPC, B, N) for r in range(NCORES)],
        axis=0,
    )  # [C, B, N]
    ybcn = ycm.transpose(1, 0, 2)  # [B, C, N]
    out = ybcn.reshape(B, C, H, WD).transpose(0, 2, 3, 1)[..., ::-1]
    return np.ascontiguousarray(out, dtype=np.float32)
